# revision 4
# baseline (speedup 1.0000x reference)
"""Multi-head causal attention with RoPE on 8 TRN2 NeuronCores.

Sharding: 4-way data parallel on batch x 2-way tensor parallel on heads
(core c -> batch c//2, head-group c%2 of 8 heads).  Each core computes its
q/k/v projections (bf16), RoPE, causal flash attention for its 8 heads, and
a row-parallel partial output projection.  No on-device collectives: the two
head-group partials per batch are summed on the host during unsharding.

Kernel layout notes:
 - x arrives host-pre-transposed as xT [1024, 2048] bf16; weights arrive
   host-transposed to [d_in, e] with q/k rows RoPE-permuted (per head:
   even components then odd components) so RoPE acts on contiguous halves.
 - q/k are roped in [s, e] layout (free-dim ops only), bounced through DRAM
   and re-loaded with the XBAR DMA-transpose into Qt/Kt [e, s] layout.
 - Scores are computed transposed, St[k, q] = Kt_h^T @ Qt_h, two heads
   packed into the PE array via row tiling (64-row strips).
 - exp(St/8) runs on ACT straight out of PSUM (logits are bounded ~|8.4|,
   so no max-subtraction is needed); the softmax denominator falls out of
   the PV matmul via a ones-column appended to V ([V|1], lhsT [128, 65]).
 - The denominator reciprocal is broadcast across partitions with a small
   DRAM-bounce DMA; odd heads' outputs are moved into their partition slot
   with an SBUF->SBUF DMA.
"""

import sys

if "/opt/trn_rl_repo" not in sys.path:
    sys.path.insert(0, "/opt/trn_rl_repo")

import numpy as np
import ml_dtypes

import concourse.bass as bass
import concourse.mybir as mybir
from concourse.bass_utils import run_bass_kernel_spmd
from concourse.tile import TileContext
from concourse.vector_clock import ScopedClock
from concourse import tile as tile_mod

bf16 = ml_dtypes.bfloat16
F32 = mybir.dt.float32
BF16 = mybir.dt.bfloat16

B, S, D = 4, 2048, 1024
H, DH = 16, 64           # total heads, head dim
HC = 8                   # heads per core
THETA = 10000.0
N_CORES = 8

# ----------------------------------------------------------------------------
# neuronxcc sync-wait-limit workarounds (this walrus build rejects >2 waits
# per instruction, and >1 on DMA pseudo-instructions).
# ----------------------------------------------------------------------------
_counter = [0]


def _patched_drain_and_barrier(self, tick_clock, wait_clock):
    nc = self.nc
    probe = nc.sync.nop(nofuse=True, hint="tail_drain_probe")
    wait_clock.add_sem_waits(probe.ins, ScopedClock({None: tick_clock.global_clock}))
    waits = []
    if probe.ins.sync_info and probe.ins.sync_info.on_wait:
        waits = list(probe.ins.sync_info.on_wait)
    if len(waits) > 1:
        probe.ins.sync_info.on_wait = waits[:1]
        for w in waits[1:]:
            nop = nc.sync.nop(nofuse=True, hint="tail_drain_split")
            si = nop.ins.sync_info
            if si is None:
                nop.ins.sync_info = mybir.SyncInfo(on_wait=[w], on_update=[])
            else:
                si.on_wait = [w]
    nc.sync.drain()
    nc.all_engine_barrier()
    assert self.sems is not None
    popped = nc._tile_sem_poison_stack.pop()
    assert popped is self._sem_poison
    nc.clear_and_free_semaphores(list(self.sems.allocated().values()))
    nc.all_engine_barrier()


tile_mod.TileContext._drain_and_barrier = _patched_drain_and_barrier


def _wait_budget(inst):
    # Observed limits in this walrus build: Drain chokes on 3 waits, DMA
    # pseudo-instructions and 3-src TensorTensor choke on 2.  One wait per
    # instruction is universally safe; excess waits ride on NoOps.
    return 1


def split_excess_waits(nc):
    for fn in nc.m.functions:
        for bb in fn.blocks:
            new_list = []
            for inst in bb.instructions:
                si = getattr(inst, "sync_info", None)
                waits = list(si.on_wait) if (si is not None and si.on_wait) else []
                budget = _wait_budget(inst)
                if len(waits) > budget:
                    extra = waits[:-budget] if budget > 0 else waits
                    for i in range(0, len(extra), 1):
                        chunk = extra[i : i + 1]
                        _counter[0] += 1
                        nop = mybir.InstNoOp(
                            name=f"I-waitsplit-{_counter[0]}", ins=[], outs=[]
                        )
                        nop.engine = inst.engine
                        nop.sync_info = mybir.SyncInfo(on_wait=chunk, on_update=[])
                        new_list.append(nop)
                    si.on_wait = waits[-budget:] if budget > 0 else []
                new_list.append(inst)
            bb.instructions[:] = new_list


# ----------------------------------------------------------------------------
# Device graph
# ----------------------------------------------------------------------------
def build_nc():
    nc = bass.Bass("TRN2", target_bir_lowering=False, debug=False,
                   num_devices=N_CORES)

    xt_ext = nc.declare_dram_parameter("xt", [D, S], BF16, isOutput=False)
    wq_ext = nc.declare_dram_parameter("wq", [D, 512], BF16, isOutput=False)
    wk_ext = nc.declare_dram_parameter("wk", [D, 512], BF16, isOutput=False)
    wv_ext = nc.declare_dram_parameter("wv", [D, 512], BF16, isOutput=False)
    wo_ext = nc.declare_dram_parameter("wo", [512, D], BF16, isOutput=False)
    cc_ext = nc.declare_dram_parameter("cc", [S, 512], BF16, isOutput=False)
    ss_ext = nc.declare_dram_parameter("ss", [S, 512], BF16, isOutput=False)
    mk_ext = nc.declare_dram_parameter("mk", [4, 128, 512], BF16, isOutput=False)
    out_ext = nc.declare_dram_parameter("out", [D, S], F32, isOutput=True)

    Exp = mybir.ActivationFunctionType.Exp
    mult = mybir.AluOpType.mult
    sub = mybir.AluOpType.subtract
    add = mybir.AluOpType.add

    with TileContext(nc) as tc:
        with (
            tc.tile_pool(name="persist", bufs=1) as pers,
            tc.tile_pool(name="dram", bufs=1, space="DRAM") as dram,
            tc.tile_pool(name="dbounce", bufs=4, space="DRAM") as dbounce,
        ):
            # persistent tiles
            vsb = pers.tile([128, 16, HC, 65], BF16, tag="vsb")
            ot = [pers.tile([128, S], BF16, tag=f"ot{p}", name=f"ot{p}") for p in range(4)]
            masks = pers.tile([128, 4, 512], BF16, tag="masks")
            wo_t = pers.tile([128, 4, D], BF16, tag="wo")
            nc.sync.dma_start(masks[:], mk_ext[:].rearrange("m k q -> k m q"))
            nc.sync.dma_start(wo_t[:], wo_ext[:].rearrange("(t p) m -> p t m", p=128))
            nc.vector.memset(vsb[:, :, :, 64:65], 1.0)

            qrot_d = dram.tile([S, 512], BF16, tag="qrot")
            krot_d = dram.tile([S, 512], BF16, tag="krot")

            # ---------------- Phase 1: projections + rope -------------------
            with (
                tc.tile_pool(name="ph1", bufs=1) as ph1,
                tc.tile_pool(name="rope", bufs=4) as rope,
                tc.tile_pool(name="ps1", bufs=4, space="PSUM") as ps1,
            ):
                xt = ph1.tile([128, 8, S], BF16, tag="xt")
                wq_t = ph1.tile([128, 8, 512], BF16, tag="wq")
                wk_t = ph1.tile([128, 8, 512], BF16, tag="wk")
                wv_t = ph1.tile([128, 8, 512], BF16, tag="wv")
                cc_t = ph1.tile([128, 16, 512], BF16, tag="cc")
                ss_t = ph1.tile([128, 16, 512], BF16, tag="ss")
                nc.sync.dma_start(xt[:], xt_ext[:].rearrange("(t p) s -> p t s", p=128))
                nc.sync.dma_start(wq_t[:], wq_ext[:].rearrange("(t p) e -> p t e", p=128))
                nc.sync.dma_start(wk_t[:], wk_ext[:].rearrange("(t p) e -> p t e", p=128))
                nc.sync.dma_start(wv_t[:], wv_ext[:].rearrange("(t p) e -> p t e", p=128))
                nc.sync.dma_start(cc_t[:], cc_ext[:].rearrange("(t p) e -> p t e", p=128))
                nc.sync.dma_start(ss_t[:], ss_ext[:].rearrange("(t p) e -> p t e", p=128))

                for s in range(16):
                    scol = slice(s * 128, (s + 1) * 128)
                    # V projection straight into the [V|1] layout
                    psv = ps1.tile([128, 512], F32, tag="psv")
                    for d in range(8):
                        nc.tensor.matmul(psv[:], lhsT=xt[:, d, scol],
                                         rhs=wv_t[:, d, :],
                                         start=(d == 0), stop=(d == 7))
                    nc.vector.tensor_copy(
                        out=vsb[:, s, :, 0:64],
                        in_=psv[:].rearrange("p (h c) -> p h c", h=HC),
                    )
                    # Q / K projections + rope
                    for w_t, rot_d in ((wq_t, qrot_d), (wk_t, krot_d)):
                        psq = ps1.tile([128, 512], F32, tag="psq")
                        for d in range(8):
                            nc.tensor.matmul(psq[:], lhsT=xt[:, d, scol],
                                             rhs=w_t[:, d, :],
                                             start=(d == 0), stop=(d == 7))
                        qsb = rope.tile([128, 512], BF16, tag="qsb")
                        nc.scalar.copy(qsb[:], psq[:])
                        t0 = rope.tile([128, 512], BF16, tag="t0")
                        t1 = rope.tile([128, 512], BF16, tag="t1")
                        nc.vector.tensor_tensor(t0[:], qsb[:], cc_t[:, s, :], mult)
                        nc.vector.tensor_tensor(t1[:], qsb[:], ss_t[:, s, :], mult)
                        qr = rope.tile([128, HC, 64], BF16, tag="qr")
                        t0v = t0[:].rearrange("p (h c) -> p h c", h=HC)
                        t1v = t1[:].rearrange("p (h c) -> p h c", h=HC)
                        nc.vector.tensor_tensor(
                            qr[:, :, 0:32], t0v[:, :, 0:32], t1v[:, :, 32:64], sub)
                        nc.vector.tensor_tensor(
                            qr[:, :, 32:64], t1v[:, :, 0:32], t0v[:, :, 32:64], add)
                        nc.sync.dma_start(
                            rot_d[scol, :], qr[:].rearrange("p h c -> p (h c)"))

            # ---------------- Phase 2: attention ----------------------------
            with (
                tc.tile_pool(name="qt", bufs=2) as qtp,
                tc.tile_pool(name="kt", bufs=2) as ktp,
                tc.tile_pool(name="est", bufs=6) as est,
                tc.tile_pool(name="epi", bufs=4) as epi,
                tc.tile_pool(name="psS", bufs=4, space="PSUM") as psS,
                tc.tile_pool(name="psP", bufs=4, space="PSUM") as psP,
            ):
                for p in range(4):
                    ecol = slice(p * 128, (p + 1) * 128)
                    qt = qtp.tile([128, S], BF16, tag="qt")
                    kt = ktp.tile([128, S], BF16, tag="kt")
                    nc.sync.dma_start_transpose(qt[:], qrot_d[:, ecol])
                    nc.sync.dma_start_transpose(kt[:], krot_d[:, ecol])
                    for j in range(4):
                        jcol = slice(j * 512, (j + 1) * 512)
                        n_k = 4 * j + 4
                        pv = [psP.tile([65, 512], F32, tag="pv", name=f"pv{p}_{j}_{k}") for k in range(2)]
                        for i in range(n_k):
                            icol = slice(i * 128, (i + 1) * 128)
                            e_ab = []
                            for hh in range(2):
                                prow = slice(hh * 64, (hh + 1) * 64)
                                pss = psS.tile([128, 512], F32, tag="pss")
                                nc.tensor.matmul(pss[:], lhsT=kt[prow, icol],
                                                 rhs=qt[prow, jcol],
                                                 start=True, stop=True)
                                e_t = est.tile([128, 512], BF16, tag="est")
                                nc.scalar.activation(e_t[:], pss[:], Exp, scale=0.125)
                                if i >= 4 * j:
                                    nc.vector.tensor_tensor(
                                        e_t[:], e_t[:], masks[:, i - 4 * j, :], mult)
                                e_ab.append(e_t)
                            for hh in range(2):
                                nc.tensor.matmul(
                                    pv[hh][:],
                                    lhsT=vsb[:, i, 2 * p + hh, :],
                                    rhs=e_ab[hh][:],
                                    start=(i == 0), stop=(i == n_k - 1))
                        for hh in range(2):
                            dstg = epi.tile([65, 512], F32, tag="dstg")
                            nc.vector.reciprocal(dstg[64:65, :], pv[hh][64:65, :])
                            db = dbounce.tile([1, 512], F32, tag="db")
                            nc.sync.dma_start(db[:], dstg[64:65, :])
                            rb = epi.tile([64, 512], F32, tag="rb")
                            nc.sync.dma_start(rb[:], db[:].to_broadcast([64, 512]))
                            if hh == 0:
                                nc.vector.tensor_tensor(
                                    ot[p][0:64, jcol], pv[hh][0:64, :], rb[:], mult)
                            else:
                                stg = epi.tile([64, 512], BF16, tag="stg")
                                nc.vector.tensor_tensor(
                                    stg[:], pv[hh][0:64, :], rb[:], mult)
                                nc.sync.dma_start(ot[p][64:128, jcol], stg[:])

            # ---------------- Phase 3: output projection --------------------
            with (
                tc.tile_pool(name="og", bufs=3) as ogp,
                tc.tile_pool(name="psO", bufs=4, space="PSUM") as psO,
            ):
                for mt in range(8):
                    mcol = slice(mt * 128, (mt + 1) * 128)
                    for sb_ in range(4):
                        scol = slice(sb_ * 512, (sb_ + 1) * 512)
                        pso = psO.tile([128, 512], F32, tag="pso")
                        for p in range(4):
                            nc.tensor.matmul(pso[:], lhsT=wo_t[:, p, mcol],
                                             rhs=ot[p][:, scol],
                                             start=(p == 0), stop=(p == 3))
                        og = ogp.tile([128, 512], F32, tag="og")
                        nc.scalar.copy(og[:], pso[:])
                        nc.sync.dma_start(out_ext[mcol, scol], og[:])

    split_excess_waits(nc)
    return nc


# ----------------------------------------------------------------------------
# Host-side input prep / unshard
# ----------------------------------------------------------------------------
def _rope_tables(token_positions):
    inv = THETA ** (-np.arange(0, DH // 2, dtype=np.float32) * 2.0 / DH)
    ang = token_positions.astype(np.float32)[:, None] * inv[None, :]   # [S, 32]
    cos, sin = np.cos(ang), np.sin(ang)
    cc = np.tile(np.concatenate([cos, cos], axis=1), (1, HC))          # [S, 512]
    ssn = np.tile(np.concatenate([sin, sin], axis=1), (1, HC))
    return cc.astype(bf16), ssn.astype(bf16)


def _perm():
    p = []
    for h in range(HC):
        base = h * DH
        p.extend(base + np.arange(0, DH, 2))
        p.extend(base + np.arange(1, DH, 2))
    return np.asarray(p)


def prep_in_maps(x, token_positions, q_w, k_w, v_w, o_w):
    x = np.asarray(x); token_positions = np.asarray(token_positions)
    q_w = np.asarray(q_w); k_w = np.asarray(k_w)
    v_w = np.asarray(v_w); o_w = np.asarray(o_w)

    cc, ssn = _rope_tables(token_positions)
    perm = _perm()
    mk = (np.arange(512)[None, None, :] >=
          (np.arange(128)[None, :, None] + 128 * np.arange(4)[:, None, None])
          ).astype(bf16)

    in_maps = []
    for c in range(N_CORES):
        b, hg = c // 2, c % 2
        esl = slice(hg * 512, (hg + 1) * 512)
        wq = q_w[esl, :][perm, :].T.astype(bf16)      # [D, 512]
        wk = k_w[esl, :][perm, :].T.astype(bf16)
        wv = v_w[esl, :].T.astype(bf16)
        wo = o_w[:, esl].T.astype(bf16)               # [512, D]
        in_maps.append({
            "xt": np.ascontiguousarray(x[b].T).astype(bf16),
            "wq": np.ascontiguousarray(wq), "wk": np.ascontiguousarray(wk),
            "wv": np.ascontiguousarray(wv), "wo": np.ascontiguousarray(wo),
            "cc": cc, "ss": ssn, "mk": mk,
        })
    return in_maps


def unshard(results):
    out = np.empty((B, S, D), dtype=np.float32)
    for b in range(B):
        out[b] = (results[2 * b]["out"] + results[2 * b + 1]["out"]).T
    return out


_nc_cache = [None]


def kernel(x, token_positions, q_w, k_w, v_w, o_w):
    if _nc_cache[0] is None:
        _nc_cache[0] = build_nc()
    nc = _nc_cache[0]
    in_maps = prep_in_maps(x, token_positions, q_w, k_w, v_w, o_w)
    res = run_bass_kernel_spmd(nc, in_maps, list(range(N_CORES)))
    return unshard(res.results)


if __name__ == "__main__":
    rng = np.random.default_rng(0)
    x = rng.standard_normal((B, S, D), dtype=np.float32)
    tp = np.arange(S, dtype=np.int32)
    sc = 1.0 / np.sqrt(D)
    ws = [rng.standard_normal((D, D), dtype=np.float32) * sc for _ in range(4)]
    out = kernel(x, tp, *ws)
    print("kernel ran, out shape", out.shape, "mean", float(np.abs(out).mean()))


# revision 7
# speedup vs baseline: 1.1502x; 1.1502x over previous
"""Multi-head causal attention with RoPE on 8 TRN2 NeuronCores.

Sharding: 4-way data parallel on batch x 2-way tensor parallel on heads
(core c -> batch c//2, head-group c%2 of 8 heads).  Each core computes its
q/k/v projections (bf16), RoPE, causal flash attention for its 8 heads, and
a row-parallel partial output projection.  No on-device collectives: the two
head-group partials per batch are summed on the host during unsharding.

Kernel layout notes:
 - x arrives host-pre-transposed as xT [1024, 2048] bf16; weights arrive
   host-transposed to [d_in, e] with q/k rows RoPE-permuted (per head:
   even components then odd components) so RoPE acts on contiguous halves.
 - q/k are roped in [s, e] layout (free-dim ops only), bounced through DRAM
   and re-loaded with the XBAR DMA-transpose into Qt/Kt [e, s] layout.
 - Scores are computed transposed, St[k, q] = Kt_h^T @ Qt_h, two heads
   packed into the PE array via row tiling (64-row strips).
 - exp(St/8) runs on ACT straight out of PSUM (logits are bounded ~|8.4|,
   so no max-subtraction is needed); the softmax denominator falls out of
   the PV matmul via a ones-column appended to V ([V|1], lhsT [128, 65]).
 - The denominator reciprocal is broadcast across partitions with a small
   DRAM-bounce DMA; odd heads' outputs are moved into their partition slot
   with an SBUF->SBUF DMA.
"""

import sys

if "/opt/trn_rl_repo" not in sys.path:
    sys.path.insert(0, "/opt/trn_rl_repo")

import numpy as np
import ml_dtypes

import concourse.bass as bass
import concourse.mybir as mybir
from concourse.bass_utils import run_bass_kernel_spmd
from concourse.tile import TileContext
from concourse.vector_clock import ScopedClock
from concourse import tile as tile_mod

bf16 = ml_dtypes.bfloat16
F32 = mybir.dt.float32
BF16 = mybir.dt.bfloat16

B, S, D = 4, 2048, 1024
H, DH = 16, 64           # total heads, head dim
HC = 8                   # heads per core
THETA = 10000.0
N_CORES = 8

# ----------------------------------------------------------------------------
# neuronxcc sync-wait-limit workarounds (this walrus build rejects >2 waits
# per instruction, and >1 on DMA pseudo-instructions).
# ----------------------------------------------------------------------------
_counter = [0]


def _patched_drain_and_barrier(self, tick_clock, wait_clock):
    nc = self.nc
    probe = nc.sync.nop(nofuse=True, hint="tail_drain_probe")
    wait_clock.add_sem_waits(probe.ins, ScopedClock({None: tick_clock.global_clock}))
    waits = []
    if probe.ins.sync_info and probe.ins.sync_info.on_wait:
        waits = list(probe.ins.sync_info.on_wait)
    if len(waits) > 1:
        probe.ins.sync_info.on_wait = waits[:1]
        for w in waits[1:]:
            nop = nc.sync.nop(nofuse=True, hint="tail_drain_split")
            si = nop.ins.sync_info
            if si is None:
                nop.ins.sync_info = mybir.SyncInfo(on_wait=[w], on_update=[])
            else:
                si.on_wait = [w]
    nc.sync.drain()
    nc.all_engine_barrier()
    assert self.sems is not None
    popped = nc._tile_sem_poison_stack.pop()
    assert popped is self._sem_poison
    nc.clear_and_free_semaphores(list(self.sems.allocated().values()))
    nc.all_engine_barrier()


tile_mod.TileContext._drain_and_barrier = _patched_drain_and_barrier


def _wait_budget(inst):
    # Observed limits in this walrus build: Drain chokes on 3 waits, DMA
    # pseudo-instructions and 3-src TensorTensor choke on 2.  One wait per
    # instruction is universally safe; excess waits ride on NoOps.
    return 1


def split_excess_waits(nc):
    for fn in nc.m.functions:
        for bb in fn.blocks:
            new_list = []
            for inst in bb.instructions:
                si = getattr(inst, "sync_info", None)
                waits = list(si.on_wait) if (si is not None and si.on_wait) else []
                budget = _wait_budget(inst)
                if len(waits) > budget:
                    extra = waits[:-budget] if budget > 0 else waits
                    for i in range(0, len(extra), 1):
                        chunk = extra[i : i + 1]
                        _counter[0] += 1
                        nop = mybir.InstNoOp(
                            name=f"I-waitsplit-{_counter[0]}", ins=[], outs=[]
                        )
                        nop.engine = inst.engine
                        nop.sync_info = mybir.SyncInfo(on_wait=chunk, on_update=[])
                        new_list.append(nop)
                    si.on_wait = waits[-budget:] if budget > 0 else []
                new_list.append(inst)
            bb.instructions[:] = new_list


# ----------------------------------------------------------------------------
# Device graph
# ----------------------------------------------------------------------------
def build_nc():
    nc = bass.Bass("TRN2", target_bir_lowering=False, debug=False,
                   num_devices=N_CORES)

    xt_ext = nc.declare_dram_parameter("xt", [D, S], BF16, isOutput=False)
    wq_ext = nc.declare_dram_parameter("wq", [D, 512], BF16, isOutput=False)
    wk_ext = nc.declare_dram_parameter("wk", [D, 512], BF16, isOutput=False)
    wv_ext = nc.declare_dram_parameter("wv", [D, 512], BF16, isOutput=False)
    wo_ext = nc.declare_dram_parameter("wo", [512, D], BF16, isOutput=False)
    cc_ext = nc.declare_dram_parameter("cc", [S, 512], BF16, isOutput=False)
    ss_ext = nc.declare_dram_parameter("ss", [S, 512], BF16, isOutput=False)
    mk_ext = nc.declare_dram_parameter("mk", [4, 128, 512], BF16, isOutput=False)
    out_ext = nc.declare_dram_parameter("out", [D, S], F32, isOutput=True)

    Exp = mybir.ActivationFunctionType.Exp
    mult = mybir.AluOpType.mult
    sub = mybir.AluOpType.subtract
    add = mybir.AluOpType.add

    with TileContext(nc) as tc:
        with (
            tc.tile_pool(name="persist", bufs=1) as pers,
            tc.tile_pool(name="dram", bufs=1, space="DRAM") as dram,
            tc.tile_pool(name="dbounce", bufs=4, space="DRAM") as dbounce,
        ):
            # persistent tiles
            vsb = pers.tile([128, 16, HC, 65], BF16, tag="vsb")
            ot = [pers.tile([128, S], BF16, tag=f"ot{p}", name=f"ot{p}") for p in range(4)]
            masks = pers.tile([128, 4, 512], BF16, tag="masks")
            wo_t = pers.tile([128, 4, D], BF16, tag="wo")
            nc.sync.dma_start(masks[:], mk_ext[:].rearrange("m k q -> k m q"))
            nc.sync.dma_start(wo_t[:], wo_ext[:].rearrange("(t p) m -> p t m", p=128))
            nc.vector.memset(vsb[:, :, :, 64:65], 1.0)

            qrot_d = dram.tile([S, 512], BF16, tag="qrot")
            krot_d = dram.tile([S, 512], BF16, tag="krot")

            # ---------------- Phase 1: projections + rope -------------------
            with (
                tc.tile_pool(name="ph1", bufs=1) as ph1,
                tc.tile_pool(name="rope", bufs=4) as rope,
                tc.tile_pool(name="ps1", bufs=4, space="PSUM") as ps1,
            ):
                xt = ph1.tile([128, 8, S], BF16, tag="xt")
                wq_t = ph1.tile([128, 8, 512], BF16, tag="wq")
                wk_t = ph1.tile([128, 8, 512], BF16, tag="wk")
                wv_t = ph1.tile([128, 8, 512], BF16, tag="wv")
                cc_t = ph1.tile([128, 16, 512], BF16, tag="cc")
                ss_t = ph1.tile([128, 16, 512], BF16, tag="ss")
                nc.sync.dma_start(xt[:], xt_ext[:].rearrange("(t p) s -> p t s", p=128))
                nc.sync.dma_start(wq_t[:], wq_ext[:].rearrange("(t p) e -> p t e", p=128))
                nc.sync.dma_start(wk_t[:], wk_ext[:].rearrange("(t p) e -> p t e", p=128))
                nc.sync.dma_start(wv_t[:], wv_ext[:].rearrange("(t p) e -> p t e", p=128))
                nc.sync.dma_start(cc_t[:], cc_ext[:].rearrange("(t p) e -> p t e", p=128))
                nc.sync.dma_start(ss_t[:], ss_ext[:].rearrange("(t p) e -> p t e", p=128))

                for s in range(16):
                    scol = slice(s * 128, (s + 1) * 128)
                    # V projection straight into the [V|1] layout
                    psv = ps1.tile([128, 512], F32, tag="psv")
                    for d in range(8):
                        nc.tensor.matmul(psv[:], lhsT=xt[:, d, scol],
                                         rhs=wv_t[:, d, :],
                                         start=(d == 0), stop=(d == 7))
                    nc.vector.tensor_copy(
                        out=vsb[:, s, :, 0:64],
                        in_=psv[:].rearrange("p (h c) -> p h c", h=HC),
                    )
                    # Q / K projections + rope
                    for w_t, rot_d in ((wq_t, qrot_d), (wk_t, krot_d)):
                        psq = ps1.tile([128, 512], F32, tag="psq")
                        for d in range(8):
                            nc.tensor.matmul(psq[:], lhsT=xt[:, d, scol],
                                             rhs=w_t[:, d, :],
                                             start=(d == 0), stop=(d == 7))
                        t0 = rope.tile([128, 512], BF16, tag="t0")
                        t1 = rope.tile([128, 512], BF16, tag="t1")
                        nc.vector.tensor_tensor(t0[:], psq[:], cc_t[:, s, :], mult)
                        nc.vector.tensor_tensor(t1[:], psq[:], ss_t[:, s, :], mult)
                        qr = rope.tile([128, HC, 64], BF16, tag="qr")
                        t0v = t0[:].rearrange("p (h c) -> p h c", h=HC)
                        t1v = t1[:].rearrange("p (h c) -> p h c", h=HC)
                        nc.vector.tensor_tensor(
                            qr[:, :, 0:32], t0v[:, :, 0:32], t1v[:, :, 32:64], sub)
                        nc.vector.tensor_tensor(
                            qr[:, :, 32:64], t1v[:, :, 0:32], t0v[:, :, 32:64], add)
                        nc.sync.dma_start(
                            rot_d[scol, :], qr[:].rearrange("p h c -> p (h c)"))

            # ---------------- Phase 2: attention ----------------------------
            with (
                tc.tile_pool(name="qt", bufs=2) as qtp,
                tc.tile_pool(name="kt", bufs=2) as ktp,
                tc.tile_pool(name="est", bufs=6) as est,
                tc.tile_pool(name="epi", bufs=4) as epi,
                tc.tile_pool(name="psS", bufs=2, space="PSUM") as psS,
                tc.tile_pool(name="psP", bufs=4, space="PSUM") as psP,
            ):
                for p in range(4):
                    ecol = slice(p * 128, (p + 1) * 128)
                    qt = qtp.tile([128, S], BF16, tag="qt")
                    kt = ktp.tile([128, S], BF16, tag="kt")
                    nc.sync.dma_start_transpose(qt[:], qrot_d[:, ecol])
                    nc.sync.dma_start_transpose(kt[:], krot_d[:, ecol])
                    for j in range(4):
                        jcol = slice(j * 512, (j + 1) * 512)
                        n_k = 4 * j + 4
                        pv = [psP.tile([65, 512], F32, tag="pv", name=f"pv{p}_{j}_{k}") for k in range(2)]

                        # scores for step i: one [128, 2, 512] psum tile, both
                        # heads side by side (2 banks), row-packed matmul pair
                        def emit_scores(i, _tiles={}):
                            icol = slice(i * 128, (i + 1) * 128)
                            pss = psS.tile([128, 2, 512], F32, tag="pss",
                                           name=f"pss{p}_{j}_{i}")
                            for hh in range(2):
                                prow = slice(hh * 64, (hh + 1) * 64)
                                nc.tensor.matmul(pss[:, hh, :],
                                                 lhsT=kt[prow, icol],
                                                 rhs=qt[prow, jcol],
                                                 start=True, stop=True)
                            return pss

                        # software pipeline: scores one step ahead of exp/PV
                        pss_cur = emit_scores(0)
                        for i in range(n_k):
                            pss_nxt = emit_scores(i + 1) if i + 1 < n_k else None
                            e_t = est.tile([128, 2, 512], BF16, tag="est",
                                           name=f"est{p}_{j}_{i}")
                            nc.scalar.activation(e_t[:], pss_cur[:], Exp, scale=0.125)
                            if i >= 4 * j:
                                for hh in range(2):
                                    nc.vector.tensor_tensor(
                                        e_t[:, hh, :], e_t[:, hh, :],
                                        masks[:, i - 4 * j, :], mult)
                            for hh in range(2):
                                nc.tensor.matmul(
                                    pv[hh][:],
                                    lhsT=vsb[:, i, 2 * p + hh, :],
                                    rhs=e_t[:, hh, :],
                                    start=(i == 0), stop=(i == n_k - 1))
                            pss_cur = pss_nxt
                        for hh in range(2):
                            dstg = epi.tile([65, 512], F32, tag="dstg")
                            nc.vector.reciprocal(dstg[64:65, :], pv[hh][64:65, :])
                            db = dbounce.tile([1, 512], F32, tag="db")
                            nc.sync.dma_start(db[:], dstg[64:65, :])
                            rb = epi.tile([64, 512], F32, tag="rb")
                            nc.sync.dma_start(rb[:], db[:].to_broadcast([64, 512]))
                            if hh == 0:
                                nc.vector.tensor_tensor(
                                    ot[p][0:64, jcol], pv[hh][0:64, :], rb[:], mult)
                            else:
                                stg = epi.tile([64, 512], BF16, tag="stg")
                                nc.vector.tensor_tensor(
                                    stg[:], pv[hh][0:64, :], rb[:], mult)
                                nc.sync.dma_start(ot[p][64:128, jcol], stg[:])

            # ---------------- Phase 3: output projection --------------------
            with (
                tc.tile_pool(name="og", bufs=3) as ogp,
                tc.tile_pool(name="psO", bufs=4, space="PSUM") as psO,
            ):
                for mt in range(8):
                    mcol = slice(mt * 128, (mt + 1) * 128)
                    for sb_ in range(4):
                        scol = slice(sb_ * 512, (sb_ + 1) * 512)
                        pso = psO.tile([128, 512], F32, tag="pso")
                        for p in range(4):
                            nc.tensor.matmul(pso[:], lhsT=wo_t[:, p, mcol],
                                             rhs=ot[p][:, scol],
                                             start=(p == 0), stop=(p == 3))
                        og = ogp.tile([128, 512], F32, tag="og")
                        nc.scalar.copy(og[:], pso[:])
                        nc.sync.dma_start(out_ext[mcol, scol], og[:])

    split_excess_waits(nc)
    return nc


# ----------------------------------------------------------------------------
# Host-side input prep / unshard
# ----------------------------------------------------------------------------
def _rope_tables(token_positions):
    inv = THETA ** (-np.arange(0, DH // 2, dtype=np.float32) * 2.0 / DH)
    ang = token_positions.astype(np.float32)[:, None] * inv[None, :]   # [S, 32]
    cos, sin = np.cos(ang), np.sin(ang)
    cc = np.tile(np.concatenate([cos, cos], axis=1), (1, HC))          # [S, 512]
    ssn = np.tile(np.concatenate([sin, sin], axis=1), (1, HC))
    return cc.astype(bf16), ssn.astype(bf16)


def _perm():
    p = []
    for h in range(HC):
        base = h * DH
        p.extend(base + np.arange(0, DH, 2))
        p.extend(base + np.arange(1, DH, 2))
    return np.asarray(p)


def prep_in_maps(x, token_positions, q_w, k_w, v_w, o_w):
    x = np.asarray(x); token_positions = np.asarray(token_positions)
    q_w = np.asarray(q_w); k_w = np.asarray(k_w)
    v_w = np.asarray(v_w); o_w = np.asarray(o_w)

    cc, ssn = _rope_tables(token_positions)
    perm = _perm()
    mk = (np.arange(512)[None, None, :] >=
          (np.arange(128)[None, :, None] + 128 * np.arange(4)[:, None, None])
          ).astype(bf16)

    in_maps = []
    for c in range(N_CORES):
        b, hg = c // 2, c % 2
        esl = slice(hg * 512, (hg + 1) * 512)
        wq = q_w[esl, :][perm, :].T.astype(bf16)      # [D, 512]
        wk = k_w[esl, :][perm, :].T.astype(bf16)
        wv = v_w[esl, :].T.astype(bf16)
        wo = o_w[:, esl].T.astype(bf16)               # [512, D]
        in_maps.append({
            "xt": np.ascontiguousarray(x[b].T).astype(bf16),
            "wq": np.ascontiguousarray(wq), "wk": np.ascontiguousarray(wk),
            "wv": np.ascontiguousarray(wv), "wo": np.ascontiguousarray(wo),
            "cc": cc, "ss": ssn, "mk": mk,
        })
    return in_maps


def unshard(results):
    out = np.empty((B, S, D), dtype=np.float32)
    for b in range(B):
        out[b] = (results[2 * b]["out"] + results[2 * b + 1]["out"]).T
    return out


_nc_cache = [None]


def kernel(x, token_positions, q_w, k_w, v_w, o_w):
    if _nc_cache[0] is None:
        _nc_cache[0] = build_nc()
    nc = _nc_cache[0]
    in_maps = prep_in_maps(x, token_positions, q_w, k_w, v_w, o_w)
    res = run_bass_kernel_spmd(nc, in_maps, list(range(N_CORES)))
    return unshard(res.results)


if __name__ == "__main__":
    rng = np.random.default_rng(0)
    x = rng.standard_normal((B, S, D), dtype=np.float32)
    tp = np.arange(S, dtype=np.int32)
    sc = 1.0 / np.sqrt(D)
    ws = [rng.standard_normal((D, D), dtype=np.float32) * sc for _ in range(4)]
    out = kernel(x, tp, *ws)
    print("kernel ran, out shape", out.shape, "mean", float(np.abs(out).mean()))
